# revision 135
# baseline (speedup 1.0000x reference)
"""EquivariantAttention Trainium2 kernel.

B=2, L=2048, D=512, H=8, HD=64 over 8 NeuronCores.
Head-parallel attention (core c owns head c, both batches), AllToAll to
sequence-shard the output projection (core c owns q-window [256c, 256c+256)).

Math notes:
  Qi . Ki = ||Q||*||K|| + (Bq Q) . (Bk K), Bq/Bk = basis[:63] rows.
  -> 64-row operands: qk = [Bq Q ; ||Q||], ks = [Bk K ; ||K||].
  The ip rows come DIRECTLY from x: lhsT is Wqk @ blockdiag(Bq.T, Bk.T),
  folded host-side (zero q/k bias fast path; numpy fallback otherwise).
  Scores are computed transposed ([k, q]); the softmax denominator comes
  from an appended ones-row in V (row 64). Softmax is max-free: with the
  global shift -20 and scale 1/8 the exp args stay in a safe bf16 range
  (no per-head centering needed).

Layout/engine strategy (cost-model driven):
  - ONE activation table for the whole program (norms use exp(0.5 ln x),
    Square/Copy ride along): a single 1.3us table load instead of 15.
  - batch 1's entire norm chain (squares/ssq/ln/exp) is deferred into
    attention(0)'s stream via an SBUF stash of raw Q;K, so the exp stream
    starts as soon as batch 0's norms exist; explicit SYNC_ONLY deps pin
    the scheduler's queue orders (P1 PE work before A0 scores, deferred
    norms behind window 1's exps, recv-0 loads behind send-1).
  - x, weights, V, exp(scores) in f16/bf16; y stored f16.
  - x: 8 DMAs per batch; batch 1's first half rides the Pool-engine
    descriptor generator to stay off HWDGE; proj/ip PSUM tiles alternate
    two banks so the ring never WARs a slow consumer; ssq borrows the
    score banks while they idle.
  - exp merged per k-tile pair ([128, 2, 512] PSUM AP); causal-trimmed
    matmuls with exact-coverage exp splits; causal triangle zeroed
    post-exp by a bf16 DVE multiply.
  - per-batch AllToAll (f16, DRAM); outproj feeds recv f16 straight into
    the PE.
"""

import sys

sys.path.insert(0, "/opt/trn_rl_repo")

import numpy as np

import concourse.bass as bass  # noqa: F401  (AP helpers)
import concourse.tile as tile
from concourse import bacc, mybir
from concourse.bass_utils import run_bass_kernel_spmd

F32 = mybir.dt.float32
F32R = mybir.dt.float32r
BF16 = mybir.dt.bfloat16
F16 = mybir.dt.float16
FP8 = mybir.dt.float8e4
EXP = mybir.ActivationFunctionType.Exp
LN = mybir.ActivationFunctionType.Ln

B, L, D, H, HD = 2, 2048, 512, 8, 64
NC = 8
LW = L // NC          # 256: per-core q-window for the output projection
NL = 4                # l-slices of 512 per batch
NK = L // 128         # 16 k-tiles per batch
NW = 4                # q-windows of 512 per batch
N_DUMMY = 18         # PE keep-warm matmuls spanning AllToAll #2


def _build_causal():
    # The kernel only uses Square/Ln/Exp/Copy/Identity, all of which live in
    # ONE activation table set: restrict the chooser to it so the program
    # performs exactly one table load (norms use sqrt(x) = exp(0.5 ln x)).
    import concourse.bacc as _bacc_mod
    _orig_tables = _bacc_mod.get_activation_tables
    # keep every entry (act_func_set_id is positional!) but empty all other
    # sets so the chooser can only ever pick the one we want
    _bacc_mod.get_activation_tables = lambda arch: {
        name: (fns if name == "natural_log_exp_and_others" else set())
        for name, fns in _orig_tables(arch).items()}
    try:
        return _build_causal_inner()
    finally:
        _bacc_mod.get_activation_tables = _orig_tables


def _build_causal_inner():
    nc = bacc.Bacc("TRN2", target_bir_lowering=False, debug=False,
                   enable_asserts=True, num_devices=NC)

    xt = nc.dram_tensor("xt", [B, D, L], F16, kind="ExternalInput")
    wqk4 = nc.dram_tensor("wqk4", [128, 512], F16, kind="ExternalInput")
    wib4 = nc.dram_tensor("wib4", [128, 512], F16, kind="ExternalInput")
    wv4 = nc.dram_tensor("wv4", [128, 256], F16, kind="ExternalInput")
    wo4 = nc.dram_tensor("wo4", [128, 2048], F16, kind="ExternalInput")
    obm = nc.dram_tensor("obm", [128, 2], F16, kind="ExternalInput")
    bv = nc.dram_tensor("bv", [128, 8 * HD], F32, kind="ExternalInput")
    bo4 = nc.dram_tensor("bo4", [128, 4], F32, kind="ExternalInput")
    tri2 = nc.dram_tensor("tri2", [128, 128], BF16, kind="ExternalInput")
    yts = nc.dram_tensor("yts", [B, D, LW], F16, kind="ExternalOutput")

    from contextlib import ExitStack
    with tile.TileContext(nc) as tc, ExitStack() as ctx:
        ec = ctx.enter_context
        const = ec(tc.tile_pool(name="const", bufs=1))
        xtp = ec(tc.tile_pool(name="xtp", bufs=1))
        qkp = ec(tc.tile_pool(name="qkp", bufs=2))
        ksp = ec(tc.tile_pool(name="ksp", bufs=2))
        sqp = ec(tc.tile_pool(name="sqp", bufs=2))
        vtp = ec(tc.tile_pool(name="vtp", bufs=2))
        expp = ec(tc.tile_pool(name="expp", bufs=6))
        uscp = ec(tc.tile_pool(name="uscp", bufs=2))
        rzp = ec(tc.tile_pool(name="rzp", bufs=1))
        zbp = ec(tc.tile_pool(name="zbp", bufs=1))
        nmp = ec(tc.tile_pool(name="nmp", bufs=2))
        rvp = ec(tc.tile_pool(name="rvp", bufs=2))
        ytp = ec(tc.tile_pool(name="ytp", bufs=2))
        dumb = ec(tc.tile_pool(name="dumb", bufs=1))
        pp = ec(tc.tile_pool(name="pp", bufs=2, space="PSUM"))      # 2 banks
        vp8 = ec(tc.tile_pool(name="vp8", bufs=1, space="PSUM"))    # 1 bank
        up = ec(tc.tile_pool(name="up", bufs=1, space="PSUM"))      # 1 bank
        sp = ec(tc.tile_pool(name="sp", bufs=2, space="PSUM"))      # 2x2 banks
        dram = ec(tc.tile_pool(name="dram", bufs=1, space="DRAM"))

        # ---- constants: sync queue feeds x (and wqk); scalar/gpsimd
        # queues take the rest so x streams with minimal HWDGE stalls
        wqk_sb = const.tile([128, 4, 128], F16)
        wib_sb = const.tile([128, 4, 128], F16)
        wv_sb = const.tile([128, 4, HD], F16)
        wo_sb = const.tile([128, 4, D], F16)
        ob_sb = const.tile([128, 2], F16)
        bv_sb = const.tile([128, 8, HD], F32)
        bo_sb = const.tile([128, 4], F32)
        tri_sb = const.tile([128, 128], BF16)
        shift_sb = const.tile([128, 1], F32)  # softmax global shift
        dum_sb = dumb.tile([128, 512], F16)  # PE warmup operand

        with tc.high_priority():
            nc.sync.dma_start(out=wqk_sb[:, :, :],
                              in_=wqk4[:, :].rearrange("p (c m) -> p c m",
                                                       c=4))
            nc.scalar.dma_start(out=wib_sb[:, :, :],
                                in_=wib4[:, :].rearrange("p (c m) -> p c m",
                                                         c=4))
            nc.scalar.dma_start(out=wv_sb[:, :, :],
                                in_=wv4[:, :].rearrange("p (c m) -> p c m",
                                                        c=4))
        nc.vector.memset(shift_sb[:, :], -20.0)
        nc.vector.memset(dum_sb[:, :], 0.125)
        # dummy activation: triggers the (single) table load while ACT idles
        warm_sb = dumb.tile([1, 1], F32)
        with nc.allow_low_precision(reason="table warm"):
            nc.scalar.activation(warm_sb[:, :], shift_sb[0:1, 0:1], EXP)

        def consts_early():
            nc.scalar.dma_start(out=ob_sb[:, :], in_=obm[:, :])
            nc.scalar.dma_start(out=bv_sb[:, :, :],
                                in_=bv[:, :].rearrange("p (j m) -> p j m", j=8))

        def consts_mid():
            # gated behind the x stream: needed only once attention starts,
            # and its transfer must not steal early DMA-engine time
            d = nc.scalar.dma_start(out=tri_sb[:, :], in_=tri2[:, :])
            d.ins.add_dependency(last_xdma[0].ins.name,
                                 mybir.DependencyInfo.SYNC_ONLY)

        def consts_late():
            d = nc.gpsimd.dma_start(out=wo_sb[:, :, :],
                                    in_=wo4[:, :].rearrange("p (c m) -> p c m",
                                                            c=4))
            d.ins.add_dependency(last_xdma[0].ins.name,
                                 mybir.DependencyInfo.SYNC_ONLY)
            nc.gpsimd.dma_start(out=bo_sb[:, :], in_=bo4[:, :])

        # ---- PE pstate warmup: keep the array busy until x arrives ----
        dum_ps = sp.tile([128, 2, 512], F32, tag="sp")
        for _ in range(6):
            nc.tensor.matmul(dum_ps[:, 0, :], dum_sb[:, 0:128], dum_sb[:, :],
                             start=True, stop=True)

        last_send = {}
        last_exp = {}
        last_sqrt = {}
        last_ssq = {}
        last_xdma = [None]
        last_ip = {}
        last_bcast = {}
        last_u = {}
        colls = {}
        deferred = {}
        send = [dram.tile([NC, HD, LW], F16, tag=f"send{b}", name=f"send{b}")
                for b in range(B)]
        recv = [dram.tile([NC, HD, LW], F16, tag=f"recv{b}", name=f"recv{b}")
                for b in range(B)]
        jobs = {}

        def phase_p(b):
            # x for this batch: one DMA per 128-feature half-chunk, in a
            # strict issue chain so the scheduler can't let batch-1 tiles
            # (or consts) steal HWDGE slots from batch-0's critical tiles
            xth = [[None] * 2 for _ in range(4)]
            with tc.high_priority():
                for h in range(2):
                    for dc in range(4):
                        t = xtp.tile([128, L // 2], F16, tag=f"xts{dc}h{h}b{b}")
                        # batch 1's first half rides the Pool-engine
                        # descriptor generator (in parallel with HWDGE);
                        # its second half queues on HWDGE behind batch 0
                        xq = nc.gpsimd if (b == 1 and h == 0) else nc.sync
                        d = xq.dma_start(
                            out=t[:, :],
                            in_=xt[b, 128 * dc:128 * (dc + 1),
                                   1024 * h:1024 * (h + 1)])
                        last_xdma[0] = d
                        xth[dc][h] = t
            qk = qkp.tile([64, L], F16, tag="qk")        # qS invariants
            ks = ksp.tile([64, L], F16, tag="ks")        # kS invariants
            sq = sqp.tile([128, L], F16, tag="sq")       # squares
            nm24 = nmp.tile([2, NL, 512], F16, tag=f"nm24b{b}", bufs=1,
                            name=f"nm24b{b}")
            qkr1 = (sqp.tile([128, L], F16, tag="qkr1", bufs=1, name="qkr1")
                    if b == 1 else None)
            lntmp = nmp.tile([2, NL, 512], F32, tag=f"lntmp{b}", bufs=1,
                             name=f"lntmp{b}")
            cnt = [0]

            def ptile():
                # proj/ip share two PSUM banks in strict rotation: each
                # alloc only ever WARs a fast consumer of two allocs ago
                tg = ("qkps", "ipps")[cnt[0] % 2]
                cnt[0] += 1
                return pp.tile([128, 512], F32, tag=tg, bufs=1,
                               name=f"p{b}_{cnt[0]}")

            for ls in range(NL):
                s = slice(512 * ls, 512 * (ls + 1))
                qk_ps = ptile()
                for dc in range(4):
                    nc.tensor.matmul(qk_ps[:, :], wqk_sb[:, dc, :],
                                     xth[dc][ls // 2][:, 512 * (ls % 2):
                                                      512 * (ls % 2 + 1)],
                                     start=(dc == 0), stop=(dc == 3))
                # invariant ip rows directly from x: lhsT is Wqk @ BD,
                # folded host-side (zero-bias fast path)
                ip_ps = ptile()
                for dc in range(4):
                    mm = nc.tensor.matmul(ip_ps[:, :], wib_sb[:, dc, :],
                                          xth[dc][ls // 2][:, 512 * (ls % 2):
                                                           512 * (ls % 2 + 1)],
                                          start=(dc == 0), stop=(dc == 3))
                last_ip[b] = mm
                if b == 0:
                    # Square sits in the same ACT table set as Exp/Ln
                    with nc.allow_low_precision(reason="f16 squares"):
                        nc.scalar.square(sq[:, s], qk_ps[:, :])
                else:
                    # batch 1 contributes ZERO pre-stream ACT work: stash
                    # raw Q;K and defer squares/ssq/norms into the stream
                    with nc.allow_low_precision(reason="f16 stash"):
                        nc.vector.tensor_copy(qkr1[:, s], qk_ps[:, :])
                with nc.allow_low_precision(reason="f16 invariants"):
                    nc.vector.tensor_copy(qk[0:63, s], ip_ps[0:63, :])
                    nc.vector.tensor_copy(ks[0:63, s], ip_ps[64:127, :])
                if b == 0:
                    # ssq borrows the (pre-attention idle) score banks
                    ssq_ps4 = sp.tile([128, 2, 512], F32, tag="sp")
                    ssq_ps = ssq_ps4[:, 0, :]
                    ssq_i = nc.tensor.matmul(ssq_ps[0:2, :], ob_sb[:, :],
                                             sq[:, s], start=True, stop=True)
                    last_ssq[b] = ssq_i
                    # sqrt(x) = exp(0.5 ln x): stays in the single table
                    with nc.allow_low_precision(reason="f16 norms"):
                        nc.scalar.activation(lntmp[:, ls, :],
                                             ssq_ps[0:2, :], LN)
                        last_sqrt[b] = nc.scalar.activation(
                            nm24[:, ls, :], lntmp[:, ls, :], EXP, scale=0.5)
            if b == 0:
                # batched norm-row moves (2 DMAs, off the sync queue)
                nc.scalar.dma_start(out=qk[63:64, :], in_=nm24[0:1, :, :])
                nc.scalar.dma_start(out=ks[63:64, :], in_=nm24[1:2, :, :])
            else:
                deferred[1] = (qkr1, nm24, lntmp, qk, ks)
            jobs[b] = (qk, ks, xth)

        def phase_v(b):
            qk, ks, xth = jobs[b]
            vt = vtp.tile([128, NK, HD + 1], BF16, tag="vt")
            first_v = [True]
            for kt in range(NK):
                j = kt % 8
                if j == 0:
                    vt8 = vp8.tile([128, 8, HD], F32, tag="vp8")
                for dc in range(4):
                    mm = nc.tensor.matmul(
                        vt8[:, j, :],
                        xth[dc][kt // 8][:, 128 * (kt % 8):
                                         128 * (kt % 8 + 1)],
                        wv_sb[:, dc, :], start=(dc == 0), stop=(dc == 3))
                    if first_v[0] and b == 1:
                        # batch 1's V stream must not overtake its own
                        # P-phase on the PE queue (the stash feeds the
                        # mid-stream deferred norms); batch 0's V runs
                        # free - the P0 chain is ring-latency-bound and
                        # everything else queues behind V0 if it waits
                        first_v[0] = False
                        mm.ins.add_dependency(
                            last_ip[b].ins.name,
                            mybir.DependencyInfo.SYNC_ONLY)
                if j == 7:
                    h8 = slice(kt - 7, kt + 1)
                    with nc.allow_low_precision(reason="bf16 V"):
                        nc.vector.tensor_add(vt[:, h8, 0:HD],
                                             vt8[:, :, :], bv_sb[:, :, :])
            with nc.allow_low_precision(reason="ones column"):
                nc.vector.memset(vt[:, :, HD:HD + 1], 1.0)
            jobs[b] = (qk, ks, vt)

        def attention(b):
            qk, ks, vt = jobs[b]
            pend = None     # deferred U-accumulation for the previous pair
            fin = None      # deferred normalization for the previous window
            first_exp = [b == 0]   # gate batch-0's exp stream on all sqrts
            first_mm = [b == 0]    # force phase_p(1) ahead of A0 on the PE

            def gate_exp(e_i):
                # no ordering constraint needed: every ACT op shares one
                # table, so norm ops interleave with the exp stream freely
                return e_i

            def emit_u(item):
                u_ps, n, p, ex, los = item
                npair = 2 * (n + 1)
                for j in range(2):
                    ki = 2 * p + j
                    w = slice(los[j], 512)
                    last_u[b] = nc.tensor.matmul(
                        u_ps[:, w], vt[:, ki, :], ex[:, j, w],
                        start=(p == 0 and j == 0),
                        stop=(p == npair - 1 and j == 1))

            def emit_fin(item):
                u_ps, n = item
                rz = rzp.tile([1, 512], F32R, tag="rz")
                zbb = zbp.tile([HD, 512], F32R, tag="zbb")
                usc = uscp.tile([HD, 512], F16, tag="usc")
                with nc.allow_low_precision(reason="f32r softmax denom"):
                    nc.vector.reciprocal(rz[:, :], u_ps[HD:HD + 1, :])
                last_bcast[b] = nc.gpsimd.partition_broadcast(zbb[:, :],
                                                              rz[:, :])
                with nc.allow_low_precision(reason="f16 payload"):
                    nc.vector.tensor_mul(usc[:, :], u_ps[0:HD, :],
                                         zbb[:, :])
                snd = nc.sync.dma_start(
                    out=send[b][2 * n:2 * n + 2, :, :].rearrange(
                        "h p c -> p h c"),
                    in_=usc[:, :].rearrange("p (h c) -> p h c", h=2))
                last_send[b] = snd

            for n in range(NW):
                if b == 0 and n == 2:
                    # mid-stream deferred batch-1 norms: DVE 4x squares
                    # from the stash, ssq matmuls through the score-bank
                    # rotation, exp(0.5 ln x) (no table swap), norm DMAs.
                    # Pinned into the ACT queue behind window 1's exps.
                    qkr1, nm24b, lnt, qk1, ks1 = deferred[1]
                    sq1 = sqp.tile([128, L], F16, tag="sq", name="sq1")
                    for g in range(2):
                        ssq_t = sp.tile([128, 2, 512], F32, tag="sp",
                                        name=f"ssqt{g}")
                        for j2 in range(2):
                            ls = 2 * g + j2
                            s1 = slice(512 * ls, 512 * (ls + 1))
                            with nc.allow_low_precision(reason="f16 sq"):
                                nc.vector.tensor_mul(sq1[:, s1],
                                                     qkr1[:, s1],
                                                     qkr1[:, s1])
                            nc.tensor.matmul(ssq_t[0:2, j2, :], ob_sb[:, :],
                                             sq1[:, s1],
                                             start=True, stop=True)
                        with nc.allow_low_precision(reason="f16 norms"):
                            ln_i = nc.scalar.activation(
                                lnt[:, 2 * g:2 * g + 2, :],
                                ssq_t[0:2, :, :], LN)
                            if g == 0:
                                ln_i.ins.add_dependency(
                                    last_exp[0].ins.name,
                                    mybir.DependencyInfo.SYNC_ONLY)
                    with nc.allow_low_precision(reason="f16 norms"):
                        nc.scalar.activation(nm24b[:, :, :],
                                             lnt[:, :, :], EXP, scale=0.5)
                    nc.scalar.dma_start(out=qk1[63:64, :],
                                        in_=nm24b[0:1, :, :])
                    nc.scalar.dma_start(out=ks1[63:64, :],
                                        in_=nm24b[1:2, :, :])
                qs = slice(512 * n, 512 * (n + 1))
                if n % 2 == 1:
                    # alternate U banks so window n+1's U matmuls never WAR
                    # window n's fin chain (which would head-of-line-block
                    # the next window's scores on the in-order PE queue).
                    # batch 0 borrows an idle projection bank; batch 1 runs
                    # after V1 so the V-accumulator bank is free
                    if b == 0:
                        u_ps = pp.tile([HD + 1, 512], F32, tag="qkps",
                                       bufs=1, name=f"u{b}_{n}")
                    else:
                        u_ps = vp8.tile([HD + 1, 512], F32, tag="vp8",
                                        name=f"u{b}_{n}")
                else:
                    u_ps = up.tile([HD + 1, 512], F32, tag="up")
                for p in range(2 * (n + 1)):
                    st = sp.tile([128, 2, 512], F32, tag="sp")
                    los = []
                    for j in range(2):
                        ki = 2 * p + j
                        lo = max(0, 128 * (ki - 4 * n))
                        los.append(lo)
                        w = slice(lo, 512)
                        mm = nc.tensor.matmul(
                            st[:, j, w],
                            ks[:, 128 * ki:128 * (ki + 1)],
                            qk[:, qs][:, w],
                            start=True, stop=True)
                        if first_mm[0] and n == 2:
                            # phase_p(1)'s PE work must complete before the
                            # back half of A0's score stream (its stash
                            # feeds the deferred norms at window 2)
                            first_mm[0] = False
                            mm.ins.add_dependency(
                                last_ip[1].ins.name,
                                mybir.DependencyInfo.SYNC_ONLY)
                    ex = expp.tile([128, 2, 512], BF16, tag="ex")
                    with nc.allow_low_precision(reason="bf16 softmax"):
                        # the exp stream starts only after ALL sqrts (both
                        # batches): each sqrt<->exp interleave costs a
                        # 1.3us ACT table reload
                        if los[0] == los[1]:
                            last_exp[b] = gate_exp(nc.scalar.activation(
                                ex[:, :, los[0]:512], st[:, :, los[0]:512],
                                EXP, scale=0.125, bias=shift_sb[:, 0:1]))
                        else:
                            # exact-coverage split (no stale PSUM reads)
                            gate_exp(nc.scalar.activation(
                                ex[:, :, los[1]:512], st[:, :, los[1]:512],
                                EXP, scale=0.125, bias=shift_sb[:, 0:1]))
                            last_exp[b] = nc.scalar.activation(
                                ex[:, 0, los[0]:los[1]],
                                st[:, 0, los[0]:los[1]], EXP,
                                scale=0.125, bias=shift_sb[:, 0:1])
                    # causal triangle: zero the upper half post-exp (bf16
                    # all-SBUF multiply runs at 4x and off the ACT path)
                    for j in range(2):
                        ki = 2 * p + j
                        if ki >= 4 * n:
                            d = slice(los[j], los[j] + 128)
                            with nc.allow_low_precision(reason="bf16 mask"):
                                nc.vector.tensor_mul(ex[:, j, d], ex[:, j, d],
                                                     tri_sb[:, :])
                    if pend is not None:
                        emit_u(pend)
                    if fin is not None:
                        emit_fin(fin)
                        fin = None
                    pend = (u_ps, n, p, ex, los)
                fin = (u_ps, n)
            emit_u(pend)
            pend = None
            emit_fin(fin)
            fin = None

        def a2a(b):
            colls[b] = nc.gpsimd.collective_compute(
                "AllToAll", mybir.AluOpType.bypass,
                replica_groups=[list(range(NC))],
                ins=[send[b].opt()], outs=[recv[b].opt()],
            )

        def outproj(b):
            # batch 0's recv loads ride the (drained) ACT queue behind
            # batch-1's send (same HWDGE) so they can't delay AllToAll #2's
            # start; they still fully overlap AllToAll #2
            rvh = rvp.tile([128, 4, LW], F16, tag="rvh")
            for j in range(2):
                q = nc.scalar if b == 0 else (nc.sync if j == 0
                                              else nc.scalar)
                # one load per 64-partition half: chunk (2dc+j, hd) lands
                # at partition 64j+hd, dc on the free axis
                d = q.dma_start(
                    out=rvh[64 * j:64 * (j + 1), :, :],
                    in_=recv[b][j:NC:2, :, :].rearrange("d h c -> h d c"))
                if b == 0:
                    d.ins.add_dependency(last_send[1].ins.name,
                                         mybir.DependencyInfo.SYNC_ONLY)
            yt = ytp.tile([128, 4, LW], F16, tag="yt")
            for dp in range(2):
                y_ps = pp.tile([128, 512], F32, tag=("qkps", "ipps")[dp],
                               bufs=1)
                for dt_ in range(2):
                    dt = 2 * dp + dt_
                    for dc in range(4):
                        nc.tensor.matmul(
                            y_ps[:, 256 * dt_:256 * (dt_ + 1)],
                            wo_sb[:, dc, 128 * dt:128 * (dt + 1)],
                            rvh[:, dc, :], start=(dc == 0), stop=(dc == 3))
                for dt_ in range(2):
                    dt = 2 * dp + dt_
                    with nc.allow_low_precision(reason="f16 output"):
                        nc.vector.tensor_scalar_add(
                            yt[:, dt, :], y_ps[:, 256 * dt_:256 * (dt_ + 1)],
                            bo_sb[:, dt:dt + 1])
            q = nc.gpsimd if b == 0 else nc.sync
            for dp in range(2):
                q.dma_start(
                    out=yts[b, 256 * dp:256 * (dp + 1), :].rearrange(
                        "(d p) c -> p d c", p=128),
                    in_=yt[:, 2 * dp:2 * dp + 2, :])

        def dummies(k, gate=None):
            d_ps = vp8.tile([128, 8, HD], F32, tag="vp8")
            for i in range(k):
                m = nc.tensor.matmul(
                    d_ps[:, 0:8, :].rearrange("p a b -> p (a b)"),
                    dum_sb[:, 0:128], dum_sb[:, :], start=True, stop=True)
                if gate is not None and i == 0:
                    m.ins.add_dependency(gate.ins.name,
                                         mybir.DependencyInfo.SYNC_ONLY)

        consts_early()
        phase_p(0)
        phase_v(0)
        phase_p(1)
        consts_mid()
        attention(0)
        consts_late()
        phase_v(1)
        a2a(0)
        attention(1)
        a2a(1)
        outproj(0)
        outproj(1)
    nc.compile()
    return nc


_CACHE = {}


def _get(causal: bool):
    assert causal
    if causal not in _CACHE:
        _CACHE[causal] = _build_causal()
    return _CACHE[causal]


def _make_w(coef):
    iu = np.triu_indices(D, k=1)
    a = np.zeros((D, D), np.float32)
    a[iu] = coef
    return a - a.T + np.eye(D, dtype=np.float32)


def _prep(x, mask, coef_q, coef_k, coef_v, coef_o,
          bias_q, bias_k, bias_v, bias_o, basis_q, basis_k):
    x = np.asarray(x, np.float32)
    mask = np.asarray(mask, np.float32)
    wq, wk, wv, wo = (_make_w(np.asarray(c, np.float32))
                      for c in (coef_q, coef_k, coef_v, coef_o))
    basis_q = np.asarray(basis_q, np.float32)
    basis_k = np.asarray(basis_k, np.float32)
    bq = np.asarray(bias_q, np.float32)
    bk = np.asarray(bias_k, np.float32)
    xtn = np.ascontiguousarray(x.transpose(0, 2, 1))
    wot = np.ascontiguousarray(wo.T)

    # causal fast path: mask[q, k] == 0 for k <= q else -1e9, and zero
    # q/k biases (the compiled program folds the basis into the projection)
    ii = np.arange(L)
    causal_ref = np.where(ii[None, :] <= ii[:, None], 0.0, -1e9).astype(np.float32)
    causal = bool(np.array_equal(mask, causal_ref))
    if not causal or np.any(bq) or np.any(bk):
        return False, None

    bf16 = mybir.dt.np(mybir.dt.bfloat16)
    # block-diag ip lhsT: out rows 0..62 = Bq Q, 64..126 = Bk K
    bd = np.zeros((128, 128), np.float32)
    bd[0:HD, 0:HD - 1] = basis_q[:HD - 1, :].T
    bd[HD:128, HD:128 - 1] = basis_k[:HD - 1, :].T
    ob = np.zeros((128, 2), np.float32)
    ob[0:HD, 0] = 1.0
    ob[HD:128, 1] = 1.0
    # causal triangle for a diagonal 128-block ([k, q]: k > q masked)
    kk = np.arange(128)
    tri2 = np.where(kk[:, None] <= kk[None, :], 1.0, 0.0).astype(np.float32)

    shared = {
        "xt": xtn.astype(np.float16),
        "obm": ob.astype(np.float16),
        "tri2": tri2.astype(bf16),
        "wo4": np.ascontiguousarray(
            wot.reshape(4, 128, D).transpose(1, 0, 2).reshape(128, 2048)
            ).astype(np.float16),
        "bo4": np.ascontiguousarray(
            np.asarray(bias_o, np.float32).reshape(4, 128).T),
    }

    in_maps = []
    for c in range(NC):
        hs = slice(HD * c, HD * (c + 1))
        m = dict(shared)
        wqkt = np.concatenate([wq[hs, :].T, wk[hs, :].T], axis=1)   # [512, 128]
        m["wqk4"] = np.ascontiguousarray(
            wqkt.reshape(4, 128, 128).transpose(1, 0, 2).reshape(
                128, 512)).astype(np.float16)
        wib = wqkt @ bd                                              # [512, 128]
        m["wib4"] = np.ascontiguousarray(
            wib.reshape(4, 128, 128).transpose(1, 0, 2).reshape(
                128, 512)).astype(np.float16)
        wvt = wv[hs, :].T                                            # [512, 64]
        m["wv4"] = np.ascontiguousarray(
            wvt.reshape(4, 128, HD).transpose(1, 0, 2).reshape(
                128, 256)).astype(np.float16)
        m["bv"] = np.ascontiguousarray(
            np.broadcast_to(np.asarray(bias_v, np.float32)[hs][None, None, :],
                            (128, 8, HD)).reshape(128, 8 * HD))
        in_maps.append(m)
    return True, in_maps


def _kernel_numpy(x, mask, coef_q, coef_k, coef_v, coef_o,
                  bias_q, bias_k, bias_v, bias_o, basis_q, basis_k):
    x = np.asarray(x, np.float64)
    wq, wk, wv, wo = (_make_w(np.asarray(c, np.float32)).astype(np.float64)
                      for c in (coef_q, coef_k, coef_v, coef_o))
    def proj(t, w, b):
        return t @ w.T + np.asarray(b, np.float64)
    def split(t):
        return t.reshape(B, L, H, HD).transpose(0, 2, 1, 3)
    Q = split(proj(x, wq, bias_q))
    Kk = split(proj(x, wk, bias_k))
    V = split(proj(x, wv, bias_v))
    def inv(t, basis):
        nrm = np.linalg.norm(t, axis=-1, keepdims=True)
        ip = np.einsum('bhld,nd->bhln', t, np.asarray(basis, np.float64))
        return np.concatenate([nrm, ip], axis=-1)[..., :HD]
    Qi = inv(Q, basis_q)
    Ki = inv(Kk, basis_k)
    s = np.einsum('bhld,bhmd->bhlm', Qi, Ki) / np.sqrt(HD) + \
        np.asarray(mask, np.float64)
    s = s - s.max(axis=-1, keepdims=True)
    p = np.exp(s)
    p /= p.sum(axis=-1, keepdims=True)
    out = np.einsum('bhlm,bhmd->bhld', p, V)
    out = out.transpose(0, 2, 1, 3).reshape(B, L, D)
    return proj(out, wo, bias_o).astype(np.float32)


def kernel(_trace=False, **inputs):
    causal, in_maps = _prep(**inputs)
    if not causal:
        return _kernel_numpy(**inputs)
    nc = _get(causal)
    res = run_bass_kernel_spmd(nc, in_maps, list(range(NC)), trace=_trace)
    y = np.empty((B, L, D), np.float32)
    for c in range(NC):
        y[:, LW * c:LW * (c + 1), :] = res.results[c]["yts"].astype(
            np.float32).transpose(0, 2, 1)
    if _trace:
        kernel._last = res
    return y


def bench(inputs, repeats=(1, 5), iters=5):
    """Kept for API compat; paired-repeat timing is unreliable under axon."""
    return -1.0, {}
